# revision 16
# baseline (speedup 1.0000x reference)
"""Trainium2 Bass kernel for nn_CliquesOutputLayer (self-contained).

kernel(**inputs) -> np.ndarray [4, 160000] float32.

Sharding: one formula per NeuronCore (F = 8 = n_cores). Host-side prep packs
the atoms table as E2[n] = [batch0|batch1|batch2|batch3] x 64 f32 (1KB rows)
so one gathered row serves all 4 batches, and pre-permutes grounding indices
into the block layout the kernel consumes. Per core, chunks of 512
groundings run: 12 indirect-DMA gathers (128 rows x 1KB, f32->f32r cast in
flight), 24 PE transposes ([128,128], batch-paired), PSUM->SBUF copies, the
192->16 sigmoid layer as two accumulating f32r matmuls per batch, a
block-diagonal 16->1 second layer, and a sigmoid + store of [4,512] outputs.
"""
from contextlib import ExitStack
from dataclasses import dataclass
import numpy as np
import concourse.bass as bass
import concourse.mybir as mybir

F32 = mybir.dt.float32
F32R = mybir.dt.float32r
I32 = mybir.dt.int32


@dataclass
class Cfg:
    n_atoms: int = 100000
    g: int = 20000
    b: int = 4
    d: int = 64
    h: int = 16
    gat_bufs: int = 6
    use_f32r_mm: bool = True
    cols_per_gather: int = 1

    @property
    def nblk(self):
        n = (self.g + 127) // 128
        return (n + 3) // 4 * 4

    @property
    def nchunk(self):
        return self.nblk // 4

    @property
    def gpad(self):
        return self.nblk * 128


def build_nc(cfg: Cfg) -> bass.Bass:
    B, D, H = cfg.b, cfg.d, cfg.h
    BD = B * D
    NB, NC = cfg.nblk, cfg.nchunk
    BH = B * H
    mmdt = F32R if cfg.use_f32r_mm else F32
    NG = cfg.gat_bufs

    nc = bass.Bass(trn_type="TRN2")
    e2 = nc.declare_dram_parameter("e2", [cfg.n_atoms, BD], F32, isOutput=False)
    gidx = nc.declare_dram_parameter("gidx", [128, NB * 3], I32, isOutput=False)
    w01 = nc.declare_dram_parameter("w01", [2 * D, H], F32, isOutput=False)
    w12 = nc.declare_dram_parameter("w12", [D, H], F32, isOutput=False)
    w2b = nc.declare_dram_parameter("w2b", [128, B], F32, isOutput=False)
    b1x = nc.declare_dram_parameter("b1x", [H, 1], F32, isOutput=False)
    b2x = nc.declare_dram_parameter("b2x", [B, 1], F32, isOutput=False)
    iden = nc.declare_dram_parameter("iden", [128, 128], F32, isOutput=False)
    o2d = nc.declare_dram_parameter("o2d", [B, cfg.gpad], F32, isOutput=True)

    with ExitStack() as ctx:
        gat = ctx.enter_context(nc.sbuf_tensor("gat", [128, NG, 12 * BD], mmdt))
        gidx_sb = ctx.enter_context(nc.sbuf_tensor("gidx_sb", [128, NB * 3], I32))
        w01_sb = ctx.enter_context(nc.sbuf_tensor("w01_sb", [2 * D, H], mmdt))
        w12_sb = ctx.enter_context(nc.sbuf_tensor("w12_sb", [D, H], mmdt))
        w2b_sb = ctx.enter_context(nc.sbuf_tensor("w2b_sb", [128, B], mmdt))
        b1_sb = ctx.enter_context(nc.sbuf_tensor("b1_sb", [H, 1], F32))
        b2_sb = ctx.enter_context(nc.sbuf_tensor("b2_sb", [B, 1], F32))
        id_sb = ctx.enter_context(nc.sbuf_tensor("id_sb", [128, 128], mmdt))
        x01 = ctx.enter_context(nc.sbuf_tensor("x01", [128, 3, 512], mmdt))
        x2 = ctx.enter_context(nc.sbuf_tensor("x2", [D, 3, 512], mmdt))
        h_sb = ctx.enter_context(nc.sbuf_tensor("h_sb", [128, 2, 512], mmdt))
        o_sb = ctx.enter_context(nc.sbuf_tensor("o_sb", [B, 2, 512], F32))
        p_t0 = ctx.enter_context(nc.psum_tensor("p_t0", [128, 1024], mmdt))
        p_t1 = ctx.enter_context(nc.psum_tensor("p_t1", [128, 1024], mmdt))
        p_t2 = ctx.enter_context(nc.psum_tensor("p_t2", [128, 1024], mmdt))
        p_h = ctx.enter_context(nc.psum_tensor("p_h", [H, 1, 512], F32))
        p_o = ctx.enter_context(nc.psum_tensor("p_o", [B, 1, 512], F32))

        io = ctx.enter_context(nc.semaphore("io"))
        wio = ctx.enter_context(nc.semaphore("wio"))
        ms_s = ctx.enter_context(nc.semaphore("ms_s"))
        gs = [ctx.enter_context(nc.semaphore(f"gs{c}")) for c in range(12)]
        st_p = [ctx.enter_context(nc.semaphore(f"st_p{i}")) for i in range(2)]
        t1_sem = ctx.enter_context(nc.semaphore("t1_sem"))
        t1b_sem = ctx.enter_context(nc.semaphore("t1b_sem"))
        t2_sem = ctx.enter_context(nc.semaphore("t2_sem"))
        cp1 = ctx.enter_context(nc.semaphore("cp1"))
        cp2 = ctx.enter_context(nc.semaphore("cp2"))
        mm_s = ctx.enter_context(nc.semaphore("mm_s"))
        mm2_s = ctx.enter_context(nc.semaphore("mm2_s"))
        hs_s = ctx.enter_context(nc.semaphore("hs_s"))
        os_s = ctx.enter_context(nc.semaphore("os_s"))
        block = ctx.enter_context(nc.Block())

        @block.sync
        def _(sync):
            sync.dma_start(out=gidx_sb[:], in_=gidx[:]).then_inc(io, 16)
            sync.dma_start(out=b1_sb[:], in_=b1x[:]).then_inc(io, 16)
            sync.dma_start(out=b2_sb[:], in_=b2x[:]).then_inc(io, 16)
            for q in range(NC):
                sync.wait_ge(os_s, q + 1)
                if q >= 2:
                    sync.wait_ge(st_p[q % 2], 16 * (q // 2))
                sync.dma_start(
                    out=o2d[:, q * 512:(q + 1) * 512], in_=o_sb[:, q % 2, :]
                ).then_inc(st_p[q % 2], 16)
            for par in range(2):
                sync.wait_ge(st_p[par], 16 * ((NC + 1 - par) // 2))

        @block.gpsimd
        def _(gpsimd):
            gpsimd.dma_start(out=w01_sb[:], in_=w01[:]).then_inc(wio, 16)
            gpsimd.dma_start(out=w12_sb[:], in_=w12[:]).then_inc(wio, 16)
            gpsimd.dma_start(out=w2b_sb[:], in_=w2b[:]).then_inc(wio, 16)
            gpsimd.dma_start(out=id_sb[:], in_=iden[:]).then_inc(wio, 16)
            gpsimd.memset(h_sb[:].bitcast(F32), 0.0).then_inc(ms_s, 1)
            gpsimd.wait_ge(io, 16 * 3)
            gpsimd.wait_ge(wio, 16 * 4)
            gpsimd.wait_ge(ms_s, 1)
            CPG = cfg.cols_per_gather
            NPC = 12 // CPG  # gather instructions per chunk
            for q in range(NC):
                if q >= NG:
                    gpsimd.wait_ge(t2_sem, (q - NG + 1) * 8)
                for gi in range(NPC):
                    if q >= 1:
                        gpsimd.wait_ge(gs[gi], 16 * q)
                    gpsimd.indirect_dma_start(
                        out=gat[:, q % NG, gi * CPG * BD:(gi + 1) * CPG * BD],
                        out_offset=None,
                        in_=e2[:],
                        in_offset=bass.IndirectOffsetOnAxis(
                            ap=gidx_sb[:, q * 12 + gi * CPG:q * 12 + (gi + 1) * CPG],
                            axis=0,
                        ),
                    ).then_inc(gs[gi], 16)
            for gi in range(NPC):
                gpsimd.wait_ge(gs[gi], 16 * NC)

        @block.tensor
        def _(tensor):
            tensor.wait_ge(io, 16 * 3)
            tensor.wait_ge(wio, 16 * 4)
            tensor.wait_ge(ms_s, 1)
            p_t = [p_t0, p_t1, p_t2]
            t_sems = [t1_sem, t1b_sem, t2_sem]
            for q in range(NC):
                # 24 batch-paired [128,128] transposes; p_t* single-buffered:
                # wait for ALL of chunk q-1's copies before reusing the banks
                if q >= 1:
                    tensor.wait_ge(cp1, 8 * q)
                    tensor.wait_ge(cp2, 4 * q)
                for j in range(4):
                    for sl in range(3):
                        tensor.wait_ge(gs[(3 * j + sl) // cfg.cols_per_gather],
                                       16 * (q + 1))
                        for hf in range(2):
                            tensor.matmul(
                                p_t[sl][:, hf * 512 + j * 128:hf * 512 + (j + 1) * 128],
                                gat[:, q % NG,
                                    (3 * j + sl) * BD + hf * 128:
                                    (3 * j + sl) * BD + (hf + 1) * 128],
                                id_sb[:], is_transpose=True,
                            ).then_inc(t_sems[sl], 1)
                for bp in range(B):
                    k = q * B + bp
                    if k >= 1:
                        tensor.wait_ge(hs_s, k)       # p_h single buf
                    tensor.wait_ge(cp1, 2 * (k + 1))
                    tensor.matmul(
                        p_h[:, 0, :], w01_sb[:], x01[:, k % 3, :],
                        start=True, stop=False,
                    )
                    tensor.wait_ge(cp2, k + 1)
                    tensor.matmul(
                        p_h[:, 0, :], w12_sb[:], x2[:, k % 3, :],
                        start=False, stop=True,
                    ).then_inc(mm_s, 1)
                if q >= 1:
                    tensor.wait_ge(os_s, q)
                tensor.wait_ge(hs_s, B * (q + 1))
                tensor.matmul(
                    p_o[:, 0, :], w2b_sb[:], h_sb[:, q % 2, :],
                    start=True, stop=True,
                ).then_inc(mm2_s, 1)
            tensor.wait_ge(mm2_s, NC)

        @block.vector
        def _(vector):
            for q in range(NC):
                vector.wait_ge(t1_sem, 8 * (q + 1))
                vector.wait_ge(t1b_sem, 8 * (q + 1))
                for bp in range(B):
                    k = q * B + bp
                    if k >= 3:
                        vector.wait_ge(mm_s, k - 2)
                    pr, pc = (bp % 2) * D, (bp // 2) * 512
                    vector.tensor_copy(
                        out=x01[0:D, k % 3, :], in_=p_t0[pr:pr + D, pc:pc + 512]
                    ).then_inc(cp1, 1)
                    vector.tensor_copy(
                        out=x01[D:2 * D, k % 3, :], in_=p_t1[pr:pr + D, pc:pc + 512]
                    ).then_inc(cp1, 1)
            vector.wait_ge(cp1, 2 * NC * B)

        @block.scalar
        def _(scalar):
            for q in range(NC):
                scalar.wait_ge(t2_sem, 8 * (q + 1))
                for bp in range(B):
                    k = q * B + bp
                    if k >= 3:
                        scalar.wait_ge(mm_s, k - 2)
                    pr, pc = (bp % 2) * D, (bp // 2) * 512
                    scalar.copy(
                        out=x2[:, k % 3, :], in_=p_t2[pr:pr + D, pc:pc + 512]
                    ).then_inc(cp2, 1)
                    scalar.wait_ge(mm_s, k + 1)
                    if k >= 2 and bp == 0:
                        scalar.wait_ge(mm2_s, q - 1)
                    scalar.activation(
                        h_sb[bp * 32:bp * 32 + H, q % 2, :], p_h[:, 0, :],
                        mybir.ActivationFunctionType.Sigmoid, bias=b1_sb[:],
                    ).then_inc(hs_s, 1)
                scalar.wait_ge(mm2_s, q + 1)
                if q >= 2:
                    scalar.wait_ge(st_p[q % 2], 16 * (q // 2))
                scalar.activation(
                    o_sb[:, q % 2, :], p_o[:, 0, :],
                    mybir.ActivationFunctionType.Sigmoid, bias=b2_sb[:],
                ).then_inc(os_s, 1)
            scalar.wait_ge(os_s, NC)

    return nc


def host_inputs(cfg: Cfg, atoms_embeddings, grounding_indices, W1, b1, W2, b2):
    B, D, H = cfg.b, cfg.d, cfg.h
    Fn = grounding_indices.shape[0]
    e2 = np.ascontiguousarray(
        np.transpose(np.asarray(atoms_embeddings, np.float32), (1, 0, 2))
    ).reshape(cfg.n_atoms, B * D)
    iden = np.eye(128, dtype=np.float32)
    maps = []
    for f in range(Fn):
        gi = np.asarray(grounding_indices[f], np.int64)
        gpadded = np.zeros((cfg.gpad, 3), np.int32)
        gpadded[:cfg.g] = gi.astype(np.int32)
        gidx = np.ascontiguousarray(
            gpadded.reshape(cfg.nblk, 128, 3).transpose(1, 0, 2)
        ).reshape(128, cfg.nblk * 3)
        w1f = np.asarray(W1[f], np.float32)
        w2f = np.asarray(W2[f], np.float32)
        w2bm = np.zeros((128, B), np.float32)
        for bp in range(B):
            w2bm[bp * 32:bp * 32 + H, bp] = w2f[:, 0]
        maps.append({
            "e2": e2,
            "gidx": gidx,
            "w01": np.ascontiguousarray(w1f[:2 * D]),
            "w12": np.ascontiguousarray(w1f[2 * D:]),
            "w2b": w2bm,
            "b1x": np.asarray(b1[f], np.float32)[:, None],
            "b2x": np.full((B, 1), np.float32(np.asarray(b2[f]).ravel()[0])),
            "iden": iden,
        })
    return maps


def assemble(cfg: Cfg, results):
    Fn = len(results)
    out = np.zeros((cfg.b, Fn * cfg.g), np.float32)
    for f in range(Fn):
        out[:, f * cfg.g:(f + 1) * cfg.g] = results[f]["o2d"][:, :cfg.g]
    return out


_RUNTIME = {}


def _get_runtime():
    if "nc" not in _RUNTIME:
        cfg = Cfg()
        _RUNTIME["cfg"] = cfg
        _RUNTIME["nc"] = build_nc(cfg)
    return _RUNTIME["cfg"], _RUNTIME["nc"]


def kernel(atoms_embeddings, grounding_indices, W1, b1, W2, b2):
    from concourse.bass_utils import run_bass_kernel_spmd

    cfg, nc = _get_runtime()
    maps = host_inputs(cfg, atoms_embeddings, grounding_indices, W1, b1, W2, b2)
    res = run_bass_kernel_spmd(nc, maps, list(range(len(maps))))
    return assemble(cfg, [res.results[i] for i in range(len(maps))]).astype(np.float32)



# revision 25
# speedup vs baseline: 1.2679x; 1.2679x over previous
"""Trainium2 Bass kernel for nn_CliquesOutputLayer (self-contained).

kernel(**inputs) -> np.ndarray [4, 160000] float32.

Sharding: one formula per NeuronCore (F = 8 = n_cores).

V3 design: the gather is one dma_gather(transpose=True) per chunk of 512
groundings (1536 rows x 512B bf16), which lands the data feature-on-partition
so the MLP consumes it directly -- no PE transposes, no PSUM evacuation.
int16 gather indices are made to fit by compacting the atoms table per
half (20 chunks x 1536 rows = 30720 <= int16 range), host-side.

Per chunk: 1 dma_gather -> 6 accumulating matmuls (block-diag W1, 192->16,
both batches of a pair on partitions) -> sigmoid [32,1024] -> 1 matmul
(block-diag W2, 16->1) -> sigmoid [2,1024] -> 2 stores of [2,512].
"""
from contextlib import ExitStack
from dataclasses import dataclass
import numpy as np
import ml_dtypes
import concourse.bass as bass
import concourse.bacc as bacc
import concourse.mybir as mybir

F32 = mybir.dt.float32
BF16 = mybir.dt.bfloat16
I16 = mybir.dt.int16

BF = ml_dtypes.bfloat16


@dataclass
class Cfg:
    n_atoms: int = 100000
    g: int = 20000
    b: int = 4
    d: int = 64
    h: int = 16
    gat_bufs: int = 6

    @property
    def nchunk(self):
        # chunks of 512 groundings
        return (self.g + 511) // 512

    @property
    def gpad(self):
        return self.nchunk * 512

    @property
    def half_chunks(self):
        return (self.nchunk + 1) // 2

    @property
    def half_rows(self):
        return self.half_chunks * 1536  # <= 30720, fits int16


def build_nc(cfg: Cfg) -> bass.Bass:
    NC = cfg.nchunk
    HC = cfg.half_chunks
    NR = cfg.half_rows
    NG = cfg.gat_bufs
    ES = 256           # row elements (4 batches x 64 dims, bf16)
    NI = 1536          # gather indices per chunk
    IW = NI // 16      # idx words per partition per chunk (96)

    nc = bacc.Bacc("TRN2")
    eA = nc.declare_dram_parameter("eA", [NR, ES], BF16, isOutput=False)
    eB = nc.declare_dram_parameter("eB", [NR, ES], BF16, isOutput=False)
    idx16 = nc.declare_dram_parameter("idx16", [128, NC * IW], I16, isOutput=False)
    w1blk = nc.declare_dram_parameter("w1blk", [128, 3 * 32], BF16, isOutput=False)
    w2blk = nc.declare_dram_parameter("w2blk", [32, 2], BF16, isOutput=False)
    b1x = nc.declare_dram_parameter("b1x", [32, 1], F32, isOutput=False)
    b2x = nc.declare_dram_parameter("b2x", [2, 1], F32, isOutput=False)
    o2d = nc.declare_dram_parameter("o2d", [4, cfg.gpad], F32, isOutput=True)

    with ExitStack() as ctx:
        gat = ctx.enter_context(nc.sbuf_tensor("gat", [128, NG, 2, NI], BF16))
        idx_sb = ctx.enter_context(nc.sbuf_tensor("idx_sb", [128, NC * IW], I16))
        w1_sb = ctx.enter_context(nc.sbuf_tensor("w1_sb", [128, 3, 32], BF16))
        w2_sb = ctx.enter_context(nc.sbuf_tensor("w2_sb", [32, 2], BF16))
        b1_sb = ctx.enter_context(nc.sbuf_tensor("b1_sb", [32, 1], F32))
        b2_sb = ctx.enter_context(nc.sbuf_tensor("b2_sb", [2, 1], F32))
        h_sig = ctx.enter_context(nc.sbuf_tensor("h_sig", [32, 2, 1024], BF16))
        o_sb = ctx.enter_context(nc.sbuf_tensor("o_sb", [2, 2, 1024], F32))
        p_h = ctx.enter_context(nc.psum_tensor("p_h", [32, 2, 1024], F32))
        p_o = ctx.enter_context(nc.psum_tensor("p_o", [2, 2, 1024], F32))

        io = ctx.enter_context(nc.semaphore("io"))
        gs = [ctx.enter_context(nc.semaphore(f"gs{c}")) for c in range(4)]
        st_p = [ctx.enter_context(nc.semaphore(f"st_p{i}")) for i in range(4)]
        mm_s = ctx.enter_context(nc.semaphore("mm_s"))
        mm2_s = ctx.enter_context(nc.semaphore("mm2_s"))
        hs_s = ctx.enter_context(nc.semaphore("hs_s"))
        os_s = ctx.enter_context(nc.semaphore("os_s"))
        block = ctx.enter_context(nc.Block())

        @block.sync
        def _(sync):
            sync.dma_start(out=idx_sb[:], in_=idx16[:]).then_inc(io, 16)
            sync.dma_start(out=w1_sb[:], in_=w1blk[:]).then_inc(io, 16)
            sync.dma_start(out=w2_sb[:], in_=w2blk[:]).then_inc(io, 16)
            sync.dma_start(out=b1_sb[:], in_=b1x[:]).then_inc(io, 16)
            sync.dma_start(out=b2_sb[:], in_=b2x[:]).then_inc(io, 16)
            for q in range(NC):
                sync.wait_ge(os_s, q + 1)
                for hf in range(2):
                    lane = (q % 2) * 2 + hf
                    if q >= 2:
                        sync.wait_ge(st_p[lane], 16 * (q // 2))
                    sync.dma_start(
                        out=o2d[2 * hf:2 * hf + 2, q * 512:(q + 1) * 512],
                        in_=o_sb[:, q % 2, hf * 512:(hf + 1) * 512],
                    ).then_inc(st_p[lane], 16)
            for par in range(2):
                for hf in range(2):
                    sync.wait_ge(st_p[par * 2 + hf],
                                 16 * ((NC + 1 - par) // 2))

        @block.gpsimd
        def _(gpsimd):
            gpsimd.wait_ge(io, 16 * 5)
            for q in range(NC):
                if q >= NG:
                    # gat buffer q-NG fully consumed by MLP1 of that chunk
                    gpsimd.wait_ge(mm_s, 2 * (q - NG + 1))
                if q >= 4:
                    gpsimd.wait_ge(gs[q % 4], 16 * (q // 4))
                gpsimd.dma_gather(
                    out_ap=gat[:, q % NG, :, :],
                    in_ap=(eA if q < HC else eB)[:],
                    idxs_ap=idx_sb[:, q * IW:(q + 1) * IW],
                    num_idxs=NI,
                    num_idxs_reg=NI,
                    elem_size=ES,
                    transpose=True,
                    single_packet=False,
                ).then_inc(gs[q % 4], 16)
            for c in range(4):
                gpsimd.wait_ge(gs[c], 16 * ((NC + 3 - c) // 4))

        @block.tensor
        def _(tensor):
            tensor.wait_ge(io, 16 * 5)
            for q in range(NC):
                # MLP2 of the previous chunk first (h_sig[q-1] ready?)
                if q >= 1:
                    tensor.wait_ge(hs_s, q)
                    if q >= 3:
                        tensor.wait_ge(os_s, q - 2)   # p_o[q-1] free
                    for hf in range(2):
                        tensor.matmul(
                            p_o[:, (q - 1) % 2, hf * 512:(hf + 1) * 512],
                            w2_sb[:],
                            h_sig[:, (q - 1) % 2, hf * 512:(hf + 1) * 512],
                            start=True, stop=True,
                        ).then_inc(mm2_s, 1 if hf == 1 else 0)
                tensor.wait_ge(gs[q % 4], 16 * (q // 4 + 1))
                if q >= 2:
                    tensor.wait_ge(hs_s, q - 1)       # p_h[q] free
                for a in range(3):
                    for hf in range(2):
                        tensor.matmul(
                            p_h[:, q % 2, hf * 512:(hf + 1) * 512],
                            w1_sb[:, a, :],
                            gat[:, q % NG, hf, a * 512:(a + 1) * 512],
                            start=(a == 0), stop=(a == 2),
                        ).then_inc(mm_s, 1 if a == 2 else 0)
            tensor.wait_ge(hs_s, NC)
            tensor.wait_ge(os_s, NC - 2)
            for hf in range(2):
                tensor.matmul(
                    p_o[:, (NC - 1) % 2, hf * 512:(hf + 1) * 512],
                    w2_sb[:],
                    h_sig[:, (NC - 1) % 2, hf * 512:(hf + 1) * 512],
                    start=True, stop=True,
                ).then_inc(mm2_s, 1 if hf == 1 else 0)

        @block.scalar
        def _(scalar):
            scalar.wait_ge(io, 16 * 5)
            for q in range(NC):
                scalar.wait_ge(mm_s, 2 * (q + 1))
                if q >= 2:
                    scalar.wait_ge(mm2_s, q - 1)      # h_sig[q] consumed
                scalar.activation(
                    h_sig[:, q % 2, :], p_h[:, q % 2, :],
                    mybir.ActivationFunctionType.Sigmoid, bias=b1_sb[:],
                ).then_inc(hs_s, 1)
                scalar.wait_ge(mm2_s, q + 1)
                if q >= 2:
                    scalar.wait_ge(st_p[(q % 2) * 2], 16 * (q // 2))
                    scalar.wait_ge(st_p[(q % 2) * 2 + 1], 16 * (q // 2))
                scalar.activation(
                    o_sb[:, q % 2, :], p_o[:, q % 2, :],
                    mybir.ActivationFunctionType.Sigmoid, bias=b2_sb[:],
                ).then_inc(os_s, 1)
            scalar.wait_ge(os_s, NC)

    nc.compile()
    return nc


def host_inputs(cfg: Cfg, atoms_embeddings, grounding_indices, W1, b1, W2, b2):
    B, D, H = cfg.b, cfg.d, cfg.h
    NC, HC, NR = cfg.nchunk, cfg.half_chunks, cfg.half_rows
    Fn = grounding_indices.shape[0]
    NI, IW = 1536, 96

    # atoms table: [n_atoms, B*D] packed rows, bf16
    e2 = np.ascontiguousarray(
        np.transpose(np.asarray(atoms_embeddings, np.float32), (1, 0, 2))
    ).reshape(cfg.n_atoms, B * D).astype(BF)

    maps = []
    for f in range(Fn):
        gi = np.asarray(grounding_indices[f], np.int64)
        gpadded = np.zeros((cfg.gpad, 3), np.int64)
        gpadded[:cfg.g] = gi
        # index stream per chunk: i = a*512 + j*128 + p -> grounding
        # q*512 + j*128 + p, atom slot a
        # stream[q, a, r] = gpadded[q*512 + r, a]
        stream = np.transpose(
            gpadded.reshape(NC, 512, 3), (0, 2, 1)
        )  # [NC, 3, 512]

        idx16 = np.zeros((128, NC * IW), np.int16)
        tabs = []
        for half in range(2):
            qlo, qhi = half * HC, min((half + 1) * HC, NC)
            atoms = stream[qlo:qhi].reshape(-1)          # [nchunks*1536]
            uniq, inv = np.unique(atoms, return_inverse=True)
            assert len(uniq) <= NR
            tab = np.zeros((NR, B * D), BF)
            tab[:len(uniq)] = e2[uniq]
            tabs.append(tab)
            loc = inv.astype(np.int16).reshape(qhi - qlo, NI)
            for qq in range(qhi - qlo):
                q = qlo + qq
                wrap = loc[qq].reshape(IW, 16).T          # [16, IW]
                idx16[:, q * IW:(q + 1) * IW] = np.tile(wrap, (8, 1))

        w1f = np.asarray(W1[f], np.float32)              # [192, 16]
        w1b = np.zeros((128, 3, 32), np.float32)
        for a in range(3):
            blk = w1f[a * 64:(a + 1) * 64]               # [64, 16]
            w1b[0:64, a, 0:16] = blk
            w1b[64:128, a, 16:32] = blk
        w2f = np.asarray(W2[f], np.float32)[:, 0]        # [16]
        w2b = np.zeros((32, 2), np.float32)
        w2b[0:16, 0] = w2f
        w2b[16:32, 1] = w2f

        maps.append({
            "eA": tabs[0],
            "eB": tabs[1],
            "idx16": idx16,
            "w1blk": w1b.reshape(128, 96).astype(BF),
            "w2blk": w2b.astype(BF),
            "b1x": np.tile(np.asarray(b1[f], np.float32), 2)[:, None],
            "b2x": np.full((2, 1), np.float32(np.asarray(b2[f]).ravel()[0])),
        })
    return maps


def assemble(cfg: Cfg, results):
    Fn = len(results)
    out = np.zeros((cfg.b, Fn * cfg.g), np.float32)
    for f in range(Fn):
        out[:, f * cfg.g:(f + 1) * cfg.g] = \
            np.asarray(results[f]["o2d"], np.float32)[:, :cfg.g]
    return out


_RUNTIME = {}


def _get_runtime():
    if "nc" not in _RUNTIME:
        cfg = Cfg()
        _RUNTIME["cfg"] = cfg
        _RUNTIME["nc"] = build_nc(cfg)
    return _RUNTIME["cfg"], _RUNTIME["nc"]


def kernel(atoms_embeddings, grounding_indices, W1, b1, W2, b2):
    from concourse.bass_utils import run_bass_kernel_spmd

    cfg, nc = _get_runtime()
    maps = host_inputs(cfg, atoms_embeddings, grounding_indices, W1, b1, W2, b2)
    res = run_bass_kernel_spmd(nc, maps, list(range(len(maps))))
    return assemble(cfg, [res.results[i] for i in range(len(maps))]).astype(np.float32)
